# revision 1
# baseline (speedup 1.0000x reference)
"""Trainium2 Bass kernel for nn_Attention_34033320854122.

Dense transformer attention block: QKV proj -> causal depthwise conv+SiLU ->
per-head RMSNorm -> partial RoPE -> causal attention -> output projection.

Sharding: tensor-parallel over the 16 heads across 8 NeuronCores (2 heads =
256 channels per core). Each core computes q/k/v for its channels (full
contraction over D), runs attention for its 2 heads, and produces a partial
output projection (outT_partial = Wo[:, cols] @ attn_cols^T). The host sums
the 8 partials and transposes.

Notes on fidelity to the reference:
- The reference negates the rotated RoPE sub-dim of BOTH q and k
  (return concat([-x_rot, x_pass])). The negation cancels exactly in
  q . k, so it is skipped.
- softmax is computed without max-subtraction: scores are O(1)-bounded
  (RMS-normed q/k, scale 1/sqrt(128)), far from fp32 exp overflow.

Matmuls run in float32r (PE's reduced-precision fp32 mode, ~13-bit
mantissa, full throughput at moving-dim >= 256). Raw fp32 bytes DMA'd into
f32r tiles behave bit-identically to the gpsimd cast-DMA path (verified).
"""

import ml_dtypes
import numpy as np

import concourse.bacc as bacc
import concourse.tile as tile
import concourse.mybir as mybir
from concourse import bass_utils
from concourse.masks import make_identity

# Problem shape (hardcoded per contract)
B, T, D = 1, 2048, 2048
H, HD = 16, 128
RD = 64
KCONV = 4
EPS = 1e-5
NCORES = 8
CPC = D // NCORES      # channels per core = 256
MPC = CPC // HD        # head tiles per core = 2
NT = 512               # free-dim tile for matmuls
NQ = T // NT           # 4 q tiles
NKC = T // HD          # 16 key chunks of 128
KD = D // 128          # 16 contraction chunks
PAD = KCONV - 1        # causal conv history

F32 = mybir.dt.float32
F32R = mybir.dt.float32r
BF16 = mybir.dt.bfloat16

_COMPILED = None


def _build():
    nc = bacc.Bacc("TRN2", target_bir_lowering=False, debug=False,
                   num_devices=NCORES)

    d = {}
    d["xT"] = nc.dram_tensor("xT", (D, T), BF16, kind="ExternalInput").ap()
    d["wqT"] = nc.dram_tensor("wqT", (D, CPC), BF16, kind="ExternalInput").ap()
    d["wkT"] = nc.dram_tensor("wkT", (D, CPC), BF16, kind="ExternalInput").ap()
    d["wvT"] = nc.dram_tensor("wvT", (D, CPC), BF16, kind="ExternalInput").ap()
    d["woT"] = nc.dram_tensor("woT", (CPC, D), F32R, kind="ExternalInput").ap()
    # trig: rows 0:64 = cos^T, rows 64:128 = sign-folded sin^T
    d["trig"] = nc.dram_tensor("trig", (128, T), F32, kind="ExternalInput").ap()
    # conv weights packed [128, proj(3), m(2), tap(4)]
    d["convw"] = nc.dram_tensor("convw", (128, 3, 2, KCONV), F32,
                                kind="ExternalInput").ap()
    # per-head norm weights [128, 2] (q, k)
    d["normw"] = nc.dram_tensor("normw", (128, 2), F32, kind="ExternalInput").ap()
    # causal mask strip [128, 896]: mask[kl, c] = 1.0 if kl <= c - 384
    d["maskb"] = nc.dram_tensor("maskb", (128, 896), F32R,
                                kind="ExternalInput").ap()
    outT = nc.dram_tensor("outT", (D, T), F32, kind="ExternalOutput").ap()

    inv_sqrt_hd = 1.0 / np.sqrt(HD)

    with tile.TileContext(nc) as tc:
        with (
            tc.tile_pool(name="consts", bufs=1) as consts,
            tc.tile_pool(name="raw", bufs=1) as rawp,
            tc.tile_pool(name="wqkv", bufs=1) as wqkvp,
            tc.tile_pool(name="final", bufs=1) as finalp,
            tc.tile_pool(name="xblk", bufs=2) as xp,
            tc.tile_pool(name="scratch", bufs=2) as scr,
            tc.tile_pool(name="exp", bufs=3) as expp,
            tc.tile_pool(name="attn", bufs=3) as attnp,
            tc.tile_pool(name="ostage", bufs=3) as ostp,
            tc.tile_pool(name="wo", bufs=2) as wop,
            tc.tile_pool(name="psacc", bufs=4, space="PSUM") as psacc,
            tc.tile_pool(name="pssum", bufs=1, space="PSUM") as pssum,
            tc.tile_pool(name="pssm", bufs=3, space="PSUM") as pssm,
        ):
            # ---- constants ----
            # cosT rows 0:64; ssin2: rows 0:32 = +sin, rows 32:64 = -sin
            # (both tiles base-partition 0 so 2-input DVE ops stay aligned)
            cosT_t = consts.tile([64, T], F32)
            nc.scalar.dma_start(cosT_t, d["trig"][0:64])
            ssin2_t = consts.tile([64, T], F32)
            nc.scalar.dma_start(ssin2_t, d["trig"][64:128])
            convw_t = consts.tile([128, 3, 2, KCONV], F32)
            nc.sync.dma_start(convw_t, d["convw"])
            normw_t = consts.tile([128, 2], F32)
            nc.sync.dma_start(normw_t, d["normw"])
            mask_t = consts.tile([128, 896], F32R)
            nc.scalar.dma_start(mask_t, d["maskb"])
            ones_f = consts.tile([128, 1], F32)
            nc.vector.memset(ones_f, 1.0)
            ones_hd = consts.tile([128, 1], F32R)   # lhsT for partition sums
            nc.vector.tensor_copy(ones_hd, ones_f)
            ones_1f = consts.tile([1, 128], F32)
            nc.vector.memset(ones_1f, 1.0)
            ones_1 = consts.tile([1, 128], F32R)    # lhsT for bcast over parts
            nc.vector.tensor_copy(ones_1, ones_1f)
            ident_f = consts.tile([128, 128], F32)
            make_identity(nc, ident_f)
            ident = consts.tile([128, 128], F32R)
            nc.vector.tensor_copy(ident, ident_f)
            eps_t = consts.tile([1, 1], F32)
            nc.vector.memset(eps_t, EPS)

            # ---- persistent buffers ----
            # raw (pre-conv) projections, padded by PAD zero cols at left
            rawq = rawp.tile([128, MPC, T + PAD], BF16)
            rawk = rawp.tile([128, MPC, T + PAD], BF16)
            rawv = rawp.tile([128, MPC, T + PAD], BF16)
            for r in (rawq, rawk, rawv):
                nc.vector.memset(r[:, :, 0:PAD], 0.0)
            # final q/k in head-transposed layout [HD, m, T]
            qfT = finalp.tile([128, MPC, T], F32R)
            kfT = finalp.tile([128, MPC, T], F32R)
            # v in natural layout per key-chunk: [t(128), m, chunk, HD]
            vtr = finalp.tile([128, MPC, NKC, HD], F32R)

            # =============== Phase bodies (emitted software-pipelined) ====
            # A(t): QKV projection for q-tile t.  B(s): conv/silu/rms/rope
            # for slice s (needs A(s) only, thanks to the causal pad).
            # C(t): attention + output projection for q-tile t (needs B(<=t)).
            w_all = wqkvp.tile([128, KD, 3, CPC], BF16)
            raws = (rawq, rawk, rawv)
            groups = [[(0, 0), (0, 1), (1, 0)], [(1, 1), (2, 0), (2, 1)]]

            def phaseA(tq, first=False):
                xb = xp.tile([128, KD, NT], BF16, name="xb")
                for k in range(KD):
                    if first:  # interleave weight-chunk loads with x(0)
                        for pi, wd in enumerate((d["wqT"], d["wkT"],
                                                 d["wvT"])):
                            deng = nc.sync if (k * 3 + pi) % 2 == 0 \
                                else nc.scalar
                            deng.dma_start(
                                w_all[:, k, pi, :],
                                wd[k * 128:(k + 1) * 128, :])
                    deng = nc.sync if k % 2 == 0 else nc.scalar
                    deng.dma_start(
                        xb[:, k, :],
                        d["xT"][k * 128:(k + 1) * 128,
                                tq * NT:(tq + 1) * NT],
                    )
                for grp in groups:
                    pst = [psacc.tile([128, NT], F32, tag="acc",
                                      name=f"acc{gi}")
                           for gi in range(3)]
                    for k in range(KD):
                        for gi, (pi, m) in enumerate(grp):
                            nc.tensor.matmul(
                                pst[gi],
                                w_all[:, k, pi, m * 128:(m + 1) * 128],
                                xb[:, k, :],
                                start=(k == 0),
                                stop=(k == KD - 1),
                            )
                    for gi, (pi, m) in enumerate(grp):
                        dst = raws[pi][:, m,
                                       PAD + tq * NT:PAD + (tq + 1) * NT]
                        nc.vector.tensor_copy(dst, pst[gi])

            def conv4(raw, pi, m, s):
                """4-tap causal depthwise conv on a 512-slice -> f32 scratch."""
                base = s * NT
                t0 = scr.tile([128, NT], F32, tag="cvA", name="cv0")
                nc.vector.tensor_scalar_mul(
                    t0, raw[:, m, base:base + NT], convw_t[:, pi, m, 0:1]
                )
                for j in (1, 2, 3):
                    t1 = scr.tile([128, NT], F32, tag=("cvB", "cvA")[j % 2],
                                  name="cvj")
                    nc.vector.scalar_tensor_tensor(
                        t1, raw[:, m, base + j:base + j + NT],
                        convw_t[:, pi, m, j:j + 1], t0,
                        mybir.AluOpType.mult, mybir.AluOpType.add,
                    )
                    t0 = t1
                return t0

            def phaseB(s):
                sl = slice(s * NT, (s + 1) * NT)
                for m in range(MPC):
                    # ---- q and k: conv, silu, rms-norm, rope ----
                    for pi, raw, fin, nwi in ((0, rawq, qfT, 0),
                                              (1, rawk, kfT, 1)):
                        cv = conv4(raw, pi, m, s)
                        sv = scr.tile([128, NT], F32, tag="silu")
                        nc.scalar.activation(
                            sv, cv, mybir.ActivationFunctionType.Silu)
                        sq = scr.tile([128, NT], F32R, tag="sq")
                        nc.scalar.activation(
                            sq, sv, mybir.ActivationFunctionType.Square)
                        ps_ss = pssm.tile([1, NT], F32, tag="sm")
                        nc.tensor.matmul(ps_ss, ones_hd, sq,
                                         start=True, stop=True)
                        rstd = scr.tile([1, NT], F32, tag="rst", name="rstd")
                        nc.scalar.activation(
                            rstd, ps_ss, mybir.ActivationFunctionType.Sqrt,
                            scale=1.0 / HD, bias=eps_t)
                        rr = scr.tile([1, NT], F32, tag="rst", name="rr")
                        nc.vector.reciprocal_approx_fast(rr, rstd)
                        ps_rb = pssm.tile([128, NT], F32, tag="sm")
                        nc.tensor.matmul(ps_rb, ones_1f, rr,
                                         start=True, stop=True)
                        qn = sv
                        nc.vector.scalar_tensor_tensor(
                            qn, sv, normw_t[:, nwi:nwi + 1], ps_rb,
                            mybir.AluOpType.mult, mybir.AluOpType.mult,
                        )
                        # rope rows 0:RD (pass-through rows RD:128):
                        # rot2[:,0] = qn_rot*cos; rot2[:,1] = rotate_half(qn)
                        # * sign-folded sin via output-offset muls.
                        rot2 = scr.tile([64, 2, NT], F32, tag="rot2")
                        nc.gpsimd.tensor_mul(rot2[0:32, 1, :], qn[32:64],
                                             ssin2_t[32:64, sl])
                        nc.gpsimd.tensor_mul(rot2[32:64, 1, :], qn[0:32],
                                             ssin2_t[0:32, sl])
                        nc.vector.tensor_mul(rot2[:, 0, :], qn[0:RD],
                                             cosT_t[:, sl])
                        nc.gpsimd.tensor_add(fin[0:RD, m, sl], rot2[:, 0, :],
                                             rot2[:, 1, :])
                        nc.scalar.copy(fin[RD:128, m, sl], qn[RD:128])
                    # ---- v: conv, silu, transpose to natural layout ----
                    cv = conv4(rawv, 2, m, s)
                    vv = scr.tile([128, NT], F32R, tag="gvB", name="vv")
                    nc.scalar.activation(
                        vv, cv, mybir.ActivationFunctionType.Silu)
                    ps_tr = pssm.tile([128, NT], F32R, tag="sm")
                    for sub in range(NT // 128):
                        nc.tensor.transpose(
                            ps_tr[:, sub * 128:(sub + 1) * 128],
                            vv[:, sub * 128:(sub + 1) * 128], ident)
                    nc.scalar.copy(
                        vtr[:, m, s * (NT // 128):(s + 1) * (NT // 128), :],
                        ps_tr.rearrange("p (s h) -> p s h", h=128))

            def phaseC(tq):
                qsl = slice(tq * NT, (tq + 1) * NT)
                attn_m = []
                for m in range(MPC):
                    nch = 4 * tq + 4
                    ps_attn = psacc.tile([128, NT], F32, tag="acc",
                                         name="ps_attn")
                    ps_sum = pssum.tile([1, NT], F32, tag="sum1",
                                        name="ps_sum")

                    def qk(tk):
                        ps_s = pssm.tile([128, NT], F32, tag="sm",
                                         name="ps_s")
                        nc.tensor.matmul(
                            ps_s, kfT[:, m, tk * 128:(tk + 1) * 128],
                            qfT[:, m, qsl], start=True, stop=True)
                        e = expp.tile([128, NT], F32R, tag="e", name="e")
                        nc.scalar.activation(
                            e, ps_s, mybir.ActivationFunctionType.Exp,
                            scale=inv_sqrt_hd)
                        dd = tk * 128 - tq * NT
                        if dd >= 0:  # diagonal chunk: causal mask
                            nc.vector.tensor_mul(
                                e, e, mask_t[:, 384 - dd:896 - dd])
                        return e

                    # software-pipeline QK ahead of PV by two chunks
                    epipe = [qk(t) for t in range(min(2, nch))]
                    for tk in range(nch):
                        if tk + 2 < nch:
                            epipe.append(qk(tk + 2))
                        e = epipe.pop(0)
                        nc.tensor.matmul(
                            ps_attn, vtr[:, m, tk, :], e,
                            start=(tk == 0), stop=(tk == nch - 1))
                        nc.tensor.matmul(
                            ps_sum, ones_hd, e,
                            start=(tk == 0), stop=(tk == nch - 1))
                    # normalize: attn^T *= 1/sumexp (broadcast over parts)
                    rr = scr.tile([1, NT], F32, tag="rst", name="rrs")
                    nc.vector.reciprocal_approx_fast(rr, ps_sum)
                    ps_rb = pssm.tile([128, NT], F32, tag="sm", name="ps_rb")
                    nc.tensor.matmul(ps_rb, ones_1f, rr, start=True,
                                     stop=True)
                    rb = scr.tile([128, NT], F32, tag="rbs")
                    nc.scalar.copy(rb, ps_rb)
                    am = attnp.tile([128, NT], F32R, tag="am", name="am")
                    nc.vector.tensor_mul(am, ps_attn, rb)
                    attn_m.append(am)
                # output projection for this q tile (wo prefetch 2 ahead)
                def wo_load(i):
                    wo_ch = wop.tile([128, 2, 128], F32R, tag="wo",
                                     name="wo_ch")
                    nc.sync.dma_start(
                        wo_ch,
                        d["woT"][:, i * 128:(i + 1) * 128].rearrange(
                            "(j p) n -> p j n", p=128))
                    return wo_ch
                wopipe = [wo_load(0), wo_load(1)]
                for i in range(D // 128):
                    if i + 2 < D // 128:
                        wopipe.append(wo_load(i + 2))
                    wo_ch = wopipe.pop(0)
                    ps_o = psacc.tile([128, NT], F32, tag="acc", name="ps_o")
                    for j in range(MPC):
                        nc.tensor.matmul(ps_o, wo_ch[:, j, :], attn_m[j],
                                         start=(j == 0), stop=(j == MPC - 1))
                    ost = ostp.tile([128, NT], F32, tag="ost", name="ost")
                    nc.vector.tensor_copy(ost, ps_o)
                    nc.sync.dma_start(outT[i * 128:(i + 1) * 128, qsl], ost)

            # pipelined emission: A two tiles ahead of B/C
            phaseA(0, first=True)
            phaseA(1)
            for t in range(NQ):
                phaseB(t)
                phaseC(t)
                if t + 2 < NQ:
                    phaseA(t + 2)

    nc.compile()
    return nc


def _prep_inputs(hidden_states, cos, sin, Wq, Wk, Wv, Wo,
                 conv_q_w, conv_k_w, conv_v_w, q_norm_w, k_norm_w):
    f = np.float32
    bf = ml_dtypes.bfloat16
    x = np.asarray(hidden_states, f)[0]            # [T, D]
    xT = np.ascontiguousarray(x.T.astype(bf))      # [D, T] bf16
    WqT = np.ascontiguousarray(np.asarray(Wq, f).T.astype(bf))
    WkT = np.ascontiguousarray(np.asarray(Wk, f).T.astype(bf))
    WvT = np.ascontiguousarray(np.asarray(Wv, f).T.astype(bf))
    WoT = np.ascontiguousarray(np.asarray(Wo, f).T)

    cosT = np.asarray(cos, f)[0].T                 # [RD, T]
    sinT = np.asarray(sin, f)[0].T
    trig = np.zeros((128, T), f)
    trig[0:RD] = cosT
    # ssin2 block (device rows 0:64): [0:32] = +sin[32:64], [32:64] = -sin[0:32]
    trig[RD:RD + 32] = sinT[32:64]
    trig[RD + 32:2 * RD] = -sinT[0:32]

    # causal mask strip: mask[kl, c] = 1.0 iff kl <= c - 384
    kl = np.arange(128, dtype=f)[:, None]
    cc = np.arange(896, dtype=f)[None, :]
    maskb = (kl <= cc - 384).astype(f)

    nw = np.zeros((128, 2), f)
    nw[:, 0] = np.asarray(q_norm_w, f)
    nw[:, 1] = np.asarray(k_norm_w, f)

    in_maps = []
    for c in range(NCORES):
        sl = slice(c * CPC, (c + 1) * CPC)
        convw = np.zeros((128, 3, 2, KCONV), f)
        for pi, cw in enumerate((conv_q_w, conv_k_w, conv_v_w)):
            convw[:, pi] = np.asarray(cw, f)[sl].reshape(MPC, 128, KCONV
                                                         ).transpose(1, 0, 2)
        in_maps.append({
            "xT": xT,
            "wqT": np.ascontiguousarray(WqT[:, sl]),
            "wkT": np.ascontiguousarray(WkT[:, sl]),
            "wvT": np.ascontiguousarray(WvT[:, sl]),
            "woT": np.ascontiguousarray(WoT[sl, :]),
            "trig": trig,
            "convw": np.ascontiguousarray(convw),
            "normw": nw,
            "maskb": maskb,
        })
    return in_maps


def kernel(hidden_states, cos, sin, Wq, Wk, Wv, Wo,
           conv_q_w, conv_k_w, conv_v_w, q_norm_w, k_norm_w,
           _trace=False):
    global _COMPILED
    if _COMPILED is None:
        _COMPILED = _build()
    nc = _COMPILED
    in_maps = _prep_inputs(hidden_states, cos, sin, Wq, Wk, Wv, Wo,
                           conv_q_w, conv_k_w, conv_v_w, q_norm_w, k_norm_w)
    res = bass_utils.run_bass_kernel_spmd(
        nc, in_maps, core_ids=list(range(NCORES)), trace=_trace)
    acc = np.zeros((D, T), np.float64)
    for r in res.results:
        acc += r["outT"]
    out = np.ascontiguousarray(acc.T.astype(np.float32))[None]
    if _trace:
        kernel._last_results = res
    return out

